# revision 5
# baseline (speedup 1.0000x reference)
"""bf16/fp8 attention on 8 trn2 NeuronCores, v3: (batch x key-half) sharding.

Core c handles batch c//2 and key-half c%2 (1024 keys), for ALL 2048 queries
of the batch. Loads q8 full (2MB fp8), k8 half (1MB fp8), vT half (2MB bf16)
= 5.26MB/core vs 7.34MB for the (batch x query-half) sharding. Each core
emits a partial accumulator [65, 2048] (rows 0-63 = V-weighted numerator^T,
row 64 = softmax denominator partial); the host sums the two key-halves,
normalizes, transposes, and adds bv.

Compute structure is the v2 one: fp8-DR qproj (psum borrowed from the scores
pool), col-tiled kproj/vproj pairs producing partition-stacked ks2 and a
base-0 vs0, row-tiled score pairs into [128, 1024] 2-bank psum tiles, one
fused exp per tile (16 total), ones-column attnV into per-query-block
accumulators streamed out as soon as each block completes.
"""

import sys

if "/opt/trn_rl_repo" not in sys.path:
    sys.path.insert(0, "/opt/trn_rl_repo")

import numpy as np

N, L, H, D = 4, 2048, 1024, 64
KSH = L // 2           # keys per core
NCORES = 8
HC = H // 128
NRNG = KSH // 512      # 2 key ranges per core
NKC = KSH // 128       # 8 key chunks per core
NQC = L // 512         # 4 query blocks


def build_bass():
    import concourse.bass as bass
    import concourse.mybir as mybir
    from concourse import bacc
    from concourse.masks import make_identity
    from concourse.tile import TileContext

    f32 = mybir.dt.float32
    bf16 = mybir.dt.bfloat16
    f8 = mybir.dt.float8e4
    DR = mybir.MatmulPerfMode.DoubleRow
    AF = mybir.ActivationFunctionType

    nc = bacc.Bacc("TRN2", target_bir_lowering=False, debug=False)
    # q8: fp8 DR pair-interleaved [128, qc, c2, j, 512] (h = c2*256+j*128+p)
    q8_d = nc.dram_tensor("q8", [128, NQC * 8 * 512], f8,
                          kind="ExternalInput").ap()
    # k8: fp8 [128, r, hc, 512] (h = hc*128 + p), keys of this half
    k8_d = nc.dram_tensor("k8", [128, NRNG * HC * 512], f8,
                          kind="ExternalInput").ap()
    vT_d = nc.dram_tensor("vT", [128, NRNG * HC * 512], bf16,
                          kind="ExternalInput").ap()
    wq8_d = nc.dram_tensor("wq8", [128, 8 * D], f8, kind="ExternalInput").ap()
    wk8_d = nc.dram_tensor("wk8", [128, HC * D], f8, kind="ExternalInput").ap()
    wv_d = nc.dram_tensor("wv", [128, HC * D], bf16, kind="ExternalInput").ap()
    b2_d = nc.dram_tensor("b2", [128, 2], f32, kind="ExternalInput").ap()
    # partial accumulator out (numerator^T rows 0-63, denominator row 64)
    out_d = nc.dram_tensor("outp", [65, L], f32, kind="ExternalOutput").ap()

    with TileContext(nc) as tc:
        with (
            tc.tile_pool(name="const", bufs=1) as const_pool,
            tc.tile_pool(name="w", bufs=1) as w_pool,
            tc.tile_pool(name="qk", bufs=1) as qk_pool,
            tc.tile_pool(name="kt", bufs=2) as kt_pool,
            tc.tile_pool(name="vt", bufs=2) as vt_pool,
            tc.tile_pool(name="vp", bufs=1) as vp_pool,
            tc.tile_pool(name="exp", bufs=10) as exp_pool,
            tc.tile_pool(name="fin", bufs=1) as fin_pool,
            tc.tile_pool(name="pj", bufs=1, space="PSUM") as pj_psum,
            tc.tile_pool(name="sc", bufs=2, space="PSUM") as sc_psum,
            tc.tile_pool(name="psv", bufs=1, space="PSUM") as psv_psum,
            tc.tile_pool(name="acc", bufs=2, space="PSUM") as acc_psum,
        ):
            # ---- small loads (scalar HWDGE): weights first ----
            wq8 = w_pool.tile([128, 8 * D], f8, tag="wq8")
            nc.scalar.dma_start(out=wq8[:], in_=wq8_d[:])
            wk8 = w_pool.tile([128, HC * D], f8, tag="wk8")
            nc.scalar.dma_start(out=wk8[:], in_=wk8_d[:])
            wv = w_pool.tile([128, HC * D], bf16, tag="wv")
            nc.scalar.dma_start(out=wv[:], in_=wv_d[:])
            b2 = const_pool.tile([128, 2], f32, tag="b2")
            nc.scalar.dma_start(out=b2[:], in_=b2_d[:])
            scr = const_pool.tile([1, 16], bf16, tag="scr")

            # constants before the chain (identity must precede gpsimd gates)
            ident = const_pool.tile([128, 128], bf16, tag="ident")
            make_identity(nc, ident[:])
            wst = const_pool.tile([128, 64], bf16, tag="wst")
            nc.vector.memset(wst[:], 0.0)
            wmv = const_pool.tile([128, 512], bf16, tag="wmv")
            nc.vector.memset(wmv[:], 0.0)
            # preload exp table set early
            scrf = const_pool.tile([1, 2], f32, tag="scrf")
            nc.scalar.activation(scrf[:], wst[0:1, 0:2], AF.Exp)

            # ---- chained big-load stream (one SWDGE queue, 4-deep) ----
            chain = []

            def chained_dma(tile, out_ap, in_ap):
                if len(chain) >= 4:
                    g = chain[len(chain) - 4]
                    nc.gpsimd.tensor_copy(
                        scr[0:1, len(chain) % 16 : len(chain) % 16 + 1],
                        g[0:1, 0:1],
                    )
                nc.gpsimd.dma_start(out=out_ap, in_=in_ap)
                chain.append(tile)

            q8v = q8_d.rearrange("p (q c j l) -> p q c j l", q=NQC, c=4, j=2)
            k8v = k8_d.rearrange("p (r h l) -> p r h l", r=NRNG, h=HC)
            vTv = vT_d.rearrange("p (r h l) -> p r h l", r=NRNG, h=HC)

            q8t, k8t, vtt = {}, {}, {}

            def load_q(qc):
                t = qk_pool.tile([128, 8 * 512], f8, tag=f"q8_{qc}",
                                 name=f"q8_{qc}")
                chained_dma(
                    t, t[:].rearrange("p (c j l) -> p c j l", c=4, j=2),
                    q8v[:, qc],
                )
                q8t[qc] = t

            def load_k(r):
                t = kt_pool.tile([128, HC * 512], f8, tag="k8", name=f"k8_{r}")
                chained_dma(
                    t, t[:].rearrange("p (h l) -> p h l", h=HC), k8v[:, r]
                )
                k8t[r] = t

            def load_v(r):
                t = vt_pool.tile([128, HC * 512], bf16, tag="vt",
                                 name=f"vt_{r}")
                chained_dma(
                    t, t[:].rearrange("p (h l) -> p h l", h=HC), vTv[:, r]
                )
                vtt[r] = t

            load_q(0)
            load_k(0)
            load_k(1)
            load_q(1)
            load_v(0)
            load_q(2)
            load_v(1)
            load_q(3)

            vp = vp_pool.tile([128, NKC * 65], bf16, tag="vp")
            nc.vector.memset(
                vp[:].rearrange("p (c e) -> p c e", e=65)[:, :, 64:65], 1.0
            )
            bq_sb, bk_sb = b2[:, 0:1], b2[:, 1:2]

            # ---- projections ----
            qd = qk_pool.tile([128, L], bf16, tag="qd")
            ks2 = qk_pool.tile([128, NRNG * 256], bf16, tag="ks2")
            vs0 = qk_pool.tile([64, NRNG * 512], bf16, tag="vs0")
            wq8v = wq8[:].rearrange("p (c j d) -> p c j d", c=4, j=2)
            wkv = wk8[:].rearrange("p (h d) -> p h d", h=HC)
            wvv = wv[:].rearrange("p (h d) -> p h d", h=HC)

            def qproj(qc):
                # acc-pool psum: keeps qproj out of both the pj serial chain
                # and the act-paced scores rotation
                ps = acc_psum.tile([65, 512], f32, tag="acc", name=f"pjq{qc}")
                qv = q8t[qc][:].rearrange("p (c j l) -> p c j l", c=4, j=2)
                for c2 in range(4):
                    nc.tensor.matmul(
                        ps[0:64, 0:512], wq8v[:, c2], qv[:, c2],
                        start=(c2 == 0), stop=(c2 == 3), perf_mode=DR,
                    )
                nc.vector.tensor_scalar(
                    qd[0:64, qc * 512 : (qc + 1) * 512], ps[0:64, 0:512],
                    bq_sb[0:64], 1.0 / 256.0,
                    mybir.AluOpType.add, mybir.AluOpType.mult,
                )
                nc.vector.tensor_copy(
                    qd[64:128, qc * 512 : (qc + 1) * 512],
                    qd[0:64, qc * 512 : (qc + 1) * 512],
                )

            def warm(n):
                for _ in range(n):
                    nc.tensor.matmul(
                        acc0[0:64, 0:512], wst[:, 0:64], wmv[:, 0:512],
                        start=True, stop=True, skip_group_check=True,
                    )

            def kproj(r):
                ps = pj_psum.tile([128, 512], f32, tag="pj", name=f"pjk{r}")
                kv = k8t[r][:].rearrange("p (h l) -> p h l", h=HC)
                for h in range(HC):
                    nc.tensor.matmul(
                        ps[0:64, 0:256], wkv[:, h], kv[:, h, 0:256],
                        start=(h == 0), stop=(h == HC - 1),
                        skip_group_check=True,
                    )
                    nc.tensor.matmul(
                        ps[64:128, 0:256], wkv[:, h], kv[:, h, 256:512],
                        start=(h == 0), stop=(h == HC - 1),
                        skip_group_check=True,
                    )
                nc.vector.tensor_scalar(
                    ks2[:, r * 256 : (r + 1) * 256], ps[:, 0:256],
                    bk_sb, 1.0 / 32.0,
                    mybir.AluOpType.add, mybir.AluOpType.mult,
                )

            def vproj(r):
                ps = pj_psum.tile([128, 512], f32, tag="pj", name=f"pjv{r}")
                vv = vtt[r][:].rearrange("p (h l) -> p h l", h=HC)
                for h in range(HC):
                    nc.tensor.matmul(
                        ps[0:64, 0:256], wvv[:, h], vv[:, h, 0:256],
                        start=(h == 0), stop=(h == HC - 1),
                        skip_group_check=True,
                    )
                    nc.tensor.matmul(
                        ps[64:128, 0:256], wvv[:, h], vv[:, h, 256:512],
                        start=(h == 0), stop=(h == HC - 1),
                        skip_group_check=True,
                    )
                nc.vector.tensor_copy(
                    vs0[:, r * 512 : r * 512 + 256], ps[0:64, 0:256]
                )
                nc.vector.tensor_copy(
                    vs0[:, r * 512 + 256 : (r + 1) * 512], ps[64:128, 0:256]
                )
                psv = psv_psum.tile([128, 512], bf16, tag="psv", name=f"psv{r}")
                for s in range(4):
                    nc.tensor.transpose(
                        psv[:, s * 128 : s * 128 + D],
                        vs0[:, r * 512 + s * 128 : r * 512 + (s + 1) * 128],
                        ident[0:64, 0:64],
                    )
                for s in range(4):
                    kc = r * 4 + s
                    nc.vector.tensor_copy(
                        vp[:, kc * 65 : kc * 65 + 64],
                        psv[:, s * 128 : s * 128 + D],
                    )

            # ---- scores + fused exp + attnV + per-block copy-out ----
            # step s: qc = s//4, r = (s//2)%2, j = s%2
            acc_tiles = {}
            es = {}
            outT = fin_pool.tile([65, L], f32, tag="outT")

            def SC(s):
                qc, r, j = s // 4, (s // 2) % 2, s % 2
                ksl = ks2[:, r * 256 + j * 128 : r * 256 + (j + 1) * 128]
                sc2 = sc_psum.tile([128, 1024], f32, tag="sc", name=f"sc{s}")
                qsl = qd[:, qc * 512 : (qc + 1) * 512]
                nc.tensor.matmul(
                    sc2[:, 0:512], ksl[0:64, :], qsl[0:64, :],
                    start=True, stop=True,
                )
                nc.tensor.matmul(
                    sc2[:, 512:1024], ksl[64:128, :], qsl[64:128, :],
                    start=True, stop=True,
                )
                e = exp_pool.tile([128, 1024], bf16, tag="e", name=f"e{s}")
                nc.scalar.activation(e[:], sc2[:], AF.Exp)
                es[s] = e

            def AV(s):
                qc, r, j = s // 4, (s // 2) % 2, s % 2
                e = es.pop(s)
                if s % 4 == 0:
                    acc_tiles[qc] = acc_psum.tile([65, 512], f32, tag="acc",
                                                  name=f"acc{qc}")
                acc = acc_tiles[qc]
                kcA, kcB = r * 4 + j, r * 4 + 2 + j
                nc.tensor.matmul(
                    acc[:, 0:512], vp[:, kcA * 65 : (kcA + 1) * 65],
                    e[:, 0:512],
                    start=(s % 4 == 0), stop=False, skip_group_check=True,
                )
                nc.tensor.matmul(
                    acc[:, 0:512], vp[:, kcB * 65 : (kcB + 1) * 65],
                    e[:, 512:1024],
                    start=False, stop=(s % 4 == 3), skip_group_check=True,
                )

            def CO(qc):
                nc.vector.tensor_copy(
                    outT[:, qc * 512 : (qc + 1) * 512], acc_tiles[qc][:, 0:512]
                )
                nc.scalar.dma_start(
                    out=out_d[:, qc * 512 : (qc + 1) * 512],
                    in_=outT[:, qc * 512 : (qc + 1) * 512],
                )

            acc0 = acc_psum.tile([65, 512], f32, tag="acc", name="warmacc")

            warm(16)
            qproj(0)
            warm(4)
            kproj(0)
            kproj(1)
            SC(0)
            SC(1)
            SC(2)
            SC(3)
            qproj(1)
            SC(4)
            SC(5)
            SC(6)
            SC(7)
            vproj(0)
            AV(0)
            AV(1)
            qproj(2)
            SC(8)
            SC(9)
            SC(10)
            SC(11)
            vproj(1)
            AV(2)
            AV(3)
            CO(0)
            AV(4)
            AV(5)
            AV(6)
            AV(7)
            CO(1)
            qproj(3)
            SC(12)
            SC(13)
            SC(14)
            SC(15)
            AV(8)
            AV(9)
            AV(10)
            AV(11)
            CO(2)
            AV(12)
            AV(13)
            AV(14)
            AV(15)
            CO(3)

    nc.compile()
    return nc


_NC_CACHE = None


def _get_nc():
    global _NC_CACHE
    if _NC_CACHE is None:
        _NC_CACHE = build_bass()
    return _NC_CACHE


def _pack_dr(arr):
    Lx = arr.shape[1]
    return np.ascontiguousarray(
        arr.reshape(4, 2, 128, Lx).transpose(2, 0, 1, 3).reshape(128, 8 * Lx)
    )


def _pack_hc(arr):
    Lx = arr.shape[1]
    return np.ascontiguousarray(
        arr.reshape(HC, 128, Lx).transpose(1, 0, 2).reshape(128, HC * Lx)
    )


def _make_in_maps(inputs):
    import ml_dtypes

    bf = ml_dtypes.bfloat16
    f8 = ml_dtypes.float8_e4m3fn

    qt = np.asarray(inputs["query"], np.float32).astype(f8).transpose(0, 2, 1)
    kt = np.asarray(inputs["key"], np.float32).astype(f8).transpose(0, 2, 1)
    vt = np.asarray(inputs["value"], np.float32).astype(bf).transpose(0, 2, 1)

    def pack_q_full(a):
        # a: [H, L] -> [128, qc, c2, j, 512]
        x = a.reshape(4, 2, 128, NQC, 512).transpose(2, 3, 0, 1, 4)
        return np.ascontiguousarray(x.reshape(128, NQC * 8 * 512))

    def pack_rng_hc(a):
        # a: [H, KSH] -> [128, r, hc, 512]
        x = a.reshape(HC, 128, NRNG, 512).transpose(1, 2, 0, 3)
        return np.ascontiguousarray(x.reshape(128, NRNG * HC * 512))

    wq8 = _pack_dr((np.asarray(inputs["Wq"], np.float32) * 32.0).astype(f8))
    wk8 = _pack_hc((np.asarray(inputs["Wk"], np.float32) * 32.0).astype(f8))
    wvp = _pack_hc(np.asarray(inputs["Wv"], np.float32).astype(bf))

    b2 = np.zeros((128, 2), np.float32)
    b2[0:64, 0] = np.asarray(inputs["bq"], np.float32) * 32.0
    b2[0:64, 1] = np.asarray(inputs["bk"], np.float32) * 32.0
    b2[64:128, 1] = b2[0:64, 1]

    q8s = [pack_q_full(qt[b]) for b in range(N)]
    in_maps = []
    for c in range(NCORES):
        b, kh = divmod(c, 2)
        in_maps.append(
            {
                "q8": q8s[b],
                "k8": pack_rng_hc(kt[b, :, kh * KSH : (kh + 1) * KSH]),
                "vT": pack_rng_hc(vt[b, :, kh * KSH : (kh + 1) * KSH]),
                "wq8": wq8,
                "wk8": wk8,
                "wv": wvp,
                "b2": b2,
            }
        )
    return in_maps


def kernel(query, key, value, Wq, bq, Wk, bk, Wv, bv):
    from concourse.bass_utils import run_bass_kernel_spmd

    in_maps = _make_in_maps(
        dict(query=query, key=key, value=value, Wq=Wq, bq=bq, Wk=Wk, bk=bk,
             Wv=Wv, bv=bv)
    )
    nc = _get_nc()
    try:
        res = run_bass_kernel_spmd(nc, in_maps, list(range(NCORES)))
    except Exception:
        res = run_bass_kernel_spmd(nc, in_maps, list(range(NCORES)))
    bvf = np.asarray(bv, np.float32)
    out = np.empty((N, L, D), np.float32)
    for b in range(N):
        o0 = np.asarray(res.results[2 * b]["outp"], np.float32)
        o1 = np.asarray(res.results[2 * b + 1]["outp"], np.float32)
        num = o0[0:64] + o1[0:64]
        den = o0[64:65] + o1[64:65]
        out[b] = (num / den).T + bvf
    return out
